# revision 4
# baseline (speedup 1.0000x reference)
"""Trainium2 Bass kernel for nn_PoolWithHole: 3x3 max-pool excluding the
center tap, zero-padded borders, clamped at 0:

    out[b,i,j] = max(0, max_{(di,dj)!=(0,0), |di|<=1, |dj|<=1} x[b,i+di,j+dj])

Sharding: pure data parallel over batch B=64 -> 8 NeuronCores x 8 images.

v2 design (vs the v1 partition-per-row kernel at 381 us):
  * fp16 end-to-end (tolerance 2e-2, fp16 rounding ~2.4e-4): halves HBM
    traffic AND unlocks the DVE 2x perf mode (0.5 cyc/elem for unit-stride
    16-bit tensor_tensor).
  * Rows live in the FREE dimension: each partition holds an 18-row x
    514-col block (16 output rows + vertical halo, 512 output cols +
    horizontal halo).  Both stencil axes are free-dim offsets, so the
    TensorE shift matmuls / PSUM evacuations of v1 disappear entirely.
  * 4 DVE maxes/output (provably minimal for the 8-cell hole stencil with
    binary elementwise max + shifts):
        v  = max(x_up, x_dn)          vertical hole pair (rows +-1)
        w1 = max(v, x)                = vertical 3-tap
        t  = max(w1_left, w1_right)   covers 6 cells at cols +-1
        out= max(t, relu(v))          relu on ScalarE folds the 0-clamp
  * Tile = 2 images x 64 row-blocks x 1 strip; 8 tiles/core.  Zero pads
    (image borders) are pre-zeroed slots in persistent X buffers, rewritten
    never (per-tile DMAs only touch the interior), so no per-tile memsets.
  * In-DMAs ride the SP HWDGE queue, out-DMAs the Activation HWDGE queue
    to avoid head-of-line blocking of next-tile loads behind an out-DMA
    that waits on compute.

GpSimd cannot help: walrus codegen rejects TensorTensor on Pool
(neuron_isa_check_opcode_on_engine fails; verified empirically).

Predicted per-core busy: DVE ~147 us, DMA ~107 us (35.7 MB @ 332 GB/s),
ScalarE ~74 us.
"""

import os
import sys

sys.path.insert(0, "/opt/trn_rl_repo")
os.environ.setdefault("MYCRO_LOCAL_CACHE", "1")

import numpy as np
from contextlib import ExitStack

import concourse.bass as bass  # noqa: F401  (registers AP machinery)
from concourse import bacc, mybir
import concourse.tile as tile
from concourse import bass_utils

F16 = mybir.dt.float16
RELU = mybir.ActivationFunctionType.Relu

_APC = None


def _ap_class():
    global _APC
    if _APC is None:
        _APC = type(
            bass.Bass("TRN2", target_bir_lowering=False)
            .alloc_sbuf_tensor("_apq", [1, 1], F16)
            .ap()
        )
    return _APC


def _mkap(base, doffset, dims):
    """Arbitrary affine AP into base's tensor: dims = [[step, count], ...]."""
    return _ap_class()(base.tensor, base.offset + doffset, dims)


N_CORES = 8
FULL_B, H, W = 64, 1024, 1024
B_LOCAL = FULL_B // N_CORES

R = 16        # output rows per partition block
RH = R + 2    # rows incl vertical halo
S = 512       # output cols per strip
SH = S + 2    # cols incl horizontal halo
NROW = RH * SH   # X buffer free elems (18*514 = 9252)
NV = R * SH      # v / w1 / t free elems (16*514 = 8224)
NO = R * S       # out free elems (8192)

_NC_CACHE: dict = {}


def build_nc(b_local: int, h: int, w: int):
    nc = bacc.Bacc(
        "TRN2",
        target_bir_lowering=False,
        debug=False,
        enable_asserts=False,
        num_devices=N_CORES,
    )
    x = nc.dram_tensor("x", [b_local, h, w], F16, kind="ExternalInput").ap()
    zrow = nc.dram_tensor("zrow", [1, SH], F16, kind="ExternalInput").ap()
    out = nc.dram_tensor("out", [b_local, h, w], F16, kind="ExternalOutput").ap()

    n_grp = b_local // 2
    n_strip = w // S

    with tile.TileContext(nc) as tc, ExitStack() as ctx:
        xp = ctx.enter_context(tc.tile_pool(name="xp", bufs=1))
        vt = ctx.enter_context(tc.tile_pool(name="vt", bufs=3))
        wp = ctx.enter_context(tc.tile_pool(name="wp", bufs=2))
        rp = ctx.enter_context(tc.tile_pool(name="rp", bufs=2))
        op_ = ctx.enter_context(tc.tile_pool(name="op", bufs=2))

        # Persistent X buffers, one per strip parity.  Border-zero slots
        # (left/right image edge column, top/bottom halo rows) are zeroed
        # once here; per-tile DMAs never overwrite them.
        xbufs = []
        for i in range(n_strip):
            Xi = xp.tile([128, NROW], F16, tag=f"Xb{i}")
            zc = i * (SH - 1)  # zero col slot: 0 (strip 0) / 513 (strip 1)
            nc.gpsimd.memset(Xi[:, zc:NROW:SH], 0.0)
            nc.gpsimd.memset(Xi[0:1, 0:SH], 0.0)       # row -1 of image A
            nc.gpsimd.memset(Xi[64:65, 0:SH], 0.0)     # row -1 of image B
            # partitions 63/127 can't host an engine op (start-partition
            # rule) -> zero their bottom-halo row via DMA from zrow.  On the
            # Activation HWDGE queue so the SP queue starts with tile 0's
            # loads immediately.
            nc.scalar.dma_start(Xi[63:64, (RH - 1) * SH : NROW], zrow[:, :])
            nc.scalar.dma_start(Xi[127:128, (RH - 1) * SH : NROW], zrow[:, :])
            xbufs.append(Xi)

        # Preload the Relu act-function table during the pipeline head so
        # the first real rv doesn't pay LoadActFuncSet (~1.3 us).
        warm = rp.tile([128, 8], F16, tag="warm")
        nc.scalar.activation(warm[0:1, 0:1], xbufs[0][0:1, 0:1], RELU)

        for g in range(n_grp):
            for s in range(n_strip):
                X = xbufs[s]
                co = 1 - s        # SBUF col slot of first loaded col
                c0 = s * (S - 1)  # DRAM first col (0 / 511)
                nl = S + 1        # 513 cols loaded
                for half in range(2):
                    b = 2 * g + half
                    P0 = 64 * half
                    xb = x[b, :, :]
                    # main: partitions P0+1..P0+62 (blocks q=1..62), 18 rows
                    Xs = X[P0 + 1 : P0 + 63, :]
                    nc.sync.dma_start(
                        _mkap(Xs, co, [[Xs.ap[0][0], 62], [SH, RH], [1, nl]]),
                        _mkap(
                            xb,
                            (R - 1) * w + c0,
                            [[R * w, 62], [w, RH], [1, nl]],
                        ),
                    )
                    # top block q=0: rows 0..16 -> row slots 1..17
                    Xt = X[P0 : P0 + 1, :]
                    nc.sync.dma_start(
                        _mkap(
                            Xt, SH + co, [[Xt.ap[0][0], 1], [SH, RH - 1], [1, nl]]
                        ),
                        xb[0 : RH - 1, c0 : c0 + nl],
                    )
                    # bottom block q=63: rows h-17..h-1 -> row slots 0..16
                    Xm = X[P0 + 63 : P0 + 64, :]
                    nc.sync.dma_start(
                        _mkap(Xm, co, [[Xm.ap[0][0], 1], [SH, RH - 1], [1, nl]]),
                        xb[h - (RH - 1) : h, c0 : c0 + nl],
                    )

                # v[r] = max(row r-1, row r+1) for the 16 center rows
                V = vt.tile([128, NV], F16, tag="vt")
                nc.vector.tensor_max(V[:, 0:NV], X[:, 0:NV], X[:, 2 * SH : NROW])
                # w1 = max(v, center row) = vertical 3-tap
                W1 = wp.tile([128, NV], F16, tag="w1")
                nc.vector.tensor_max(W1[:, 0:NV], V[:, 0:NV], X[:, SH : SH + NV])
                # rv = relu(v) on the 512 output cols (ScalarE, off DVE)
                RV = rp.tile([128, NO], F16, tag="rv")
                Vb = V[:, :]
                nc.scalar.activation(
                    RV[:, 0:NO],
                    _mkap(Vb, 1, [[Vb.ap[0][0], 128], [SH, R], [1, S]]),
                    RELU,
                )
                # t = horizontal hole pair of w1 (flat; inter-row slots unused)
                T = vt.tile([128, NV], F16, tag="vt")
                nc.vector.tensor_max(T[:, 0 : NV - 2], W1[:, 0 : NV - 2], W1[:, 2:NV])
                O = op_.tile([128, NO], F16, tag="o")
                Tb = T[:, :]
                ob = out[2 * g, :, :]
                last = g == n_grp - 1 and s == n_strip - 1
                # Last tile: split final op + out-DMA into row halves so the
                # drain DMA overlaps the second half's compute.
                halves = 2 if last else 1
                rh_ = R // halves
                for k in range(halves):
                    f0 = k * rh_ * S
                    nc.vector.tensor_max(
                        O[:, f0 : f0 + rh_ * S],
                        _mkap(
                            Tb,
                            k * rh_ * SH,
                            [[Tb.ap[0][0], 128], [SH, rh_], [1, S]],
                        ),
                        RV[:, f0 : f0 + rh_ * S],
                    )
                    # out-DMA on the Activation HWDGE queue (not SP) so it
                    # can't head-of-line block the next tile's in-DMAs.
                    nc.scalar.dma_start(
                        _mkap(
                            ob,
                            s * S + k * rh_ * w,
                            [[h * w, 2], [R * w, 64], [w, rh_], [1, S]],
                        ),
                        O[:, f0 : f0 + rh_ * S],
                    )

    nc.compile()
    return nc


def _get_nc(b_local: int, h: int, w: int):
    key = (b_local, h, w)
    if key not in _NC_CACHE:
        _NC_CACHE[key] = build_nc(b_local, h, w)
    return _NC_CACHE[key]


def _in_maps(x16: np.ndarray, b_local: int):
    zrow = np.zeros((1, SH), dtype=np.float16)
    return [
        {
            "x": np.ascontiguousarray(x16[i * b_local : (i + 1) * b_local]),
            "zrow": zrow,
        }
        for i in range(N_CORES)
    ]


def kernel(x: np.ndarray, **_unused) -> np.ndarray:
    """Full-input entry point: x [64,1024,1024] fp32 -> out same shape."""
    x = np.asarray(x)
    assert x.shape == (FULL_B, H, W), x.shape
    x16 = x.astype(np.float16)
    nc = _get_nc(B_LOCAL, H, W)
    res = bass_utils.run_bass_kernel_spmd(
        nc, _in_maps(x16, B_LOCAL), core_ids=list(range(N_CORES))
    )
    return np.concatenate(
        [np.asarray(r["out"]).astype(np.float32) for r in res.results], axis=0
    )
